# revision 13
# baseline (speedup 1.0000x reference)
"""YOLOv1 loss kernel for Trainium2, data-parallel over 8 NeuronCores.

Full inputs: pred [16384,30,7,7] f32, labels [16384,30,7,7] f32 -> scalar f32.

Sharding: batch 16384 -> 8 cores x 2048 rows. Per core the kernel streams
pred [2048,1470] and a host-packed labels tensor [2048,1225] (channels 0-4
and 10-29; channels 5-9 are exact duplicates / unused in the reference),
computes the per-cell loss fully on-chip and reduces to [128, NCHUNK]
partial sums. Host sums the 8*128*NCHUNK partials and divides by B.

Math notes (all equivalent to the reference up to f32 rounding):
  - The grid offsets m,n cancel inside the IOU (equal shift of both boxes),
    and scaling all coords by 7 cancels in inter/union, so
    lo = x - 3.5w, hi = x + 3.5w, inter_raw = 49*inter,
    den = 49*(a1+ag) - inter_raw, iou = inter_raw/den.
  - a = w*h equals the reference's (x2-x1)*(y2-y1).
  - den >= 49*ag - ulp > 0 always (labels w,h >= 0.05), so the where() guard
    in the reference is unnecessary: inter==0 already gives iou = 0/den = 0.
  - obj = labels[:,4] exactly (conf is exactly 0/1).
  - inner = U2 + resp*(U1-U2) + cls with U1 = 5c1 + o1 + 0.5o2,
    U2 = 5c2 + o2 + 0.5o1; cell = obj*(inner - sph) + sph,
    sph = 0.5*(p4^2+p9^2).
"""

import numpy as np

import concourse.bass as bass
import concourse.mybir as mybir
import concourse.tile as tile
from concourse import bacc
from concourse.bass_utils import run_bass_kernel_spmd

F32 = mybir.dt.float32
OP = mybir.AluOpType
AF = mybir.ActivationFunctionType

NCORES = 8
B = 16384
BLOC = B // NCORES        # 2048 rows per core
P = 128                   # SBUF partitions
K = 4                     # 128-row blocks processed per chunk
NBLK = BLOC // P          # 16
NCHUNK = NBLK // K        # 4
PREDW = 30 * 49           # 1470, host-permuted channel order (see PERM)
LABW = 29 * 49            # 1421: [lx lx ly ly lw lw lh lh obj cls*20]
W = K * 49                # 196: one channel across the K blocks

# host-side pred channel permutation: pairs the two boxes so every
# per-box op is one contiguous 3D access pattern:
# [x1 x2 y1 y2 w1 w2 h1 h2 c1 c2 cls...]
PERM = [0, 5, 1, 6, 2, 7, 3, 8, 4, 9] + list(range(10, 30))

SQ5 = float(np.float32(np.sqrt(5.0)))
ISQ2 = float(np.float32(np.sqrt(0.5)))


def _body(tc, pred_ap, labs_ap, out_ap):
    nc = tc.nc
    nv = nc.vector
    na = nc.scalar
    ng = nc.gpsimd

    # DRAM views: row index = chunk*K*P + blk*P + p ; DMA iterates [p, k, f].
    pred_r = pred_ap.rearrange("(c k p) f -> c p k f", c=NCHUNK, k=K, p=P)
    labs_r = labs_ap.rearrange("(c k p) f -> c p k f", c=NCHUNK, k=K, p=P)

    import contextlib
    ctx = contextlib.ExitStack()
    with ctx:
        inp = ctx.enter_context(tc.tile_pool(name="inp", bufs=2))
        med = ctx.enter_context(tc.tile_pool(name="med", bufs=1))
        sml = ctx.enter_context(tc.tile_pool(name="sml", bufs=2))
        opool = ctx.enter_context(tc.tile_pool(name="opool", bufs=1))

        acc = opool.tile([P, NCHUNK], F32)

        for c in range(NCHUNK):
            PT = inp.tile([P, K * PREDW], F32, tag="PT")
            LT = inp.tile([P, K * LABW], F32, tag="LT")
            nc.sync.dma_start(
                PT[:].rearrange("p (k f) -> p k f", k=K), pred_r[c])
            nc.sync.dma_start(
                LT[:].rearrange("p (k f) -> p k f", k=K), labs_r[c])

            # paired pred layout (PERM): [x1 x2 y1 y2 w1 w2 h1 h2 c1 c2 cls]
            PT3 = PT[:].rearrange("p (k f) -> p k f", k=K)
            LT3 = LT[:].rearrange("p (k f) -> p k f", k=K)

            p_xy = PT3[:, :, 0:196]       # x1 x2 y1 y2
            p_wh = PT3[:, :, 196:392]     # w1 w2 h1 h2
            p_w = PT3[:, :, 196:294]
            p_h = PT3[:, :, 294:392]
            p_cf = PT3[:, :, 392:490]     # c1 c2
            p_cls = PT3[:, :, 490:1470]
            # labels: [lx lx ly ly | lw lw lh lh | obj | cls]
            l_xy = LT3[:, :, 0:196]
            l_wh = LT3[:, :, 196:392]
            l_w = LT3[:, :, 196:294]
            l_h = LT3[:, :, 294:392]
            l_obj = LT3[:, :, 392:441]
            l_cls = LT3[:, :, 441:1421]

            def t2(name, cols, pool=med, dt=F32):
                # tile with 3D view [p, K, cols]
                t = pool.tile([P, K * cols], dt, tag=name)
                return t, t[:].rearrange("p (k f) -> p k f", k=K)

            # ---- boxes: lo = xy - 3.5*wh, hi = xy + 3.5*wh (coords x7) ----
            _, lo_p = t2("lo_p", 196)
            _, hi_p = t2("hi_p", 196)
            _, lo_g = t2("lo_g", 196)
            _, hi_g = t2("hi_g", 196)
            nv.scalar_tensor_tensor(lo_p, p_wh, -3.5, p_xy, OP.mult, OP.add)
            nv.scalar_tensor_tensor(hi_p, p_wh, 3.5, p_xy, OP.mult, OP.add)
            nv.scalar_tensor_tensor(lo_g, l_wh, -3.5, l_xy, OP.mult, OP.add)
            nv.scalar_tensor_tensor(hi_g, l_wh, 3.5, l_xy, OP.mult, OP.add)

            # ---- areas (unscaled, both gt copies): a = w*h ----
            _, aa = t2("aa", 98)     # a1 a2
            _, ag = t2("ag", 98)     # ag ag
            nv.tensor_tensor(aa, p_w, p_h, OP.mult)
            nv.tensor_tensor(ag, l_w, l_h, OP.mult)
            _, ss = t2("ss", 98)     # a_k + ag
            nv.tensor_tensor(ss, aa, ag, OP.add)

            # ---- intersection ----
            _, mx = t2("mx", 196)
            _, mn = t2("mn", 196)
            nv.tensor_tensor(mx, lo_p, lo_g, OP.max)
            nv.tensor_tensor(mn, hi_p, hi_g, OP.min)
            _, dd = t2("dd", 196)
            nv.tensor_tensor(dd, mn, mx, OP.subtract)
            na.activation(dd, dd, AF.Relu)
            _, ii = t2("ii", 98)     # inter_raw (x49): i1 i2
            nv.tensor_tensor(ii, dd[:, :, 0:98], dd[:, :, 98:196], OP.mult)

            # ---- iou = inter_raw / (49*(a+ag) - inter_raw) ----
            _, dn = t2("dn", 98)
            nv.scalar_tensor_tensor(dn, ss, 49.0, ii, OP.mult, OP.subtract)
            _, rc = t2("rc", 98)
            nv.reciprocal(rc, dn)
            _, io = t2("io", 98)
            nv.tensor_tensor(io, ii, rc, OP.mult)

            _, resp = t2("resp", 49, sml, dt=mybir.dt.int32)
            nv.tensor_tensor(resp, io[:, :, 0:49], io[:, :, 49:98], OP.is_ge)

            # ---- conf terms: objc_k = (p_conf_k - iou_k)^2 ----
            _, dcp = t2("dcp", 98)
            nv.tensor_tensor(dcp, p_cf, io, OP.subtract)
            na.activation(dcp, dcp, AF.Square)   # -> objc1 objc2

            # ---- coor terms (x5 folded into squares) ----
            _, dxy = t2("dxy", 196)
            nv.tensor_tensor(dxy, p_xy, l_xy, OP.subtract)
            na.activation(dxy, dxy, AF.Square, scale=SQ5)  # 5*(dxy)^2
            _, sp = t2("sp", 196)
            na.activation(sp, p_wh, AF.Sqrt)
            _, sl = t2("sl", 196)
            na.activation(sl, l_wh, AF.Sqrt)
            _, ee = t2("ee", 196)
            nv.tensor_tensor(ee, sp, sl, OP.subtract)
            na.activation(ee, ee, AF.Square, scale=SQ5)    # 5*(e)^2
            nv.tensor_tensor(dxy, dxy, ee, OP.add)         # g (in-place)
            _, cc = t2("cc", 98)
            nv.tensor_tensor(cc, dxy[:, :, 0:98], dxy[:, :, 98:196],
                             OP.add)                        # 5*coor1, 5*coor2

            # ---- cls = sum_c (p_c - l_c)^2 over 20 channels ----
            _, dk = t2("dk", 980)
            ng.tensor_tensor(dk, p_cls, l_cls, OP.subtract)
            na.activation(dk, dk, AF.Square)
            _, u1 = t2("u1", 490)
            ng.tensor_tensor(u1, dk[:, :, 0:490], dk[:, :, 490:980], OP.add)
            _, u2 = t2("u2", 196)
            ng.tensor_tensor(u2, u1[:, :, 0:196], u1[:, :, 196:392], OP.add)
            _, u3 = t2("u3", 98, sml)
            nv.tensor_tensor(u3, u2[:, :, 0:98], u2[:, :, 98:196], OP.add)
            _, u4 = t2("u4", 49, sml)
            nv.tensor_tensor(u4, u3[:, :, 0:49], u3[:, :, 49:98], OP.add)
            _, u5 = t2("u5", 49, sml)
            nv.tensor_tensor(u5, u1[:, :, 392:441], u1[:, :, 441:490], OP.add)
            _, cls = t2("cls", 49, sml)
            nv.tensor_tensor(cls, u4, u5, OP.add)

            # ---- combine: inner = sel(resp, U1, U2) + cls ----
            objc1 = dcp[:, :, 0:49]
            objc2 = dcp[:, :, 49:98]
            _, u1a = t2("u1a", 49, sml)
            nv.scalar_tensor_tensor(u1a, objc2, 0.5, objc1, OP.mult, OP.add)
            _, U1 = t2("U1", 49, sml)
            nv.tensor_tensor(U1, u1a, cc[:, :, 0:49], OP.add)
            _, u2a = t2("u2a", 49, sml)
            nv.scalar_tensor_tensor(u2a, objc1, 0.5, objc2, OP.mult, OP.add)
            _, U2 = t2("U2", 49, sml)
            nv.tensor_tensor(U2, u2a, cc[:, :, 49:98], OP.add)
            selU_t, selU = t2("selU", 49, sml)
            na.activation(selU, U2, AF.Copy)
            nv.copy_predicated(selU_t[:], resp, U1)
            _, inner = t2("inner", 49, sml)
            nv.tensor_tensor(inner, selU, cls, OP.add)

            # ---- cell = obj ? inner : 0.5*(c1^2+c2^2), then reduce ----
            _, hp = t2("hp", 98)
            na.activation(hp, p_cf, AF.Square, scale=ISQ2)  # 0.5*conf^2
            obj_t, obj_v = t2("obj", 49, sml, dt=mybir.dt.int32)
            na.activation(obj_v, l_obj, AF.Copy)
            cell_t, cell = t2("cell", 49, sml)
            nv.tensor_tensor(cell, hp[:, :, 0:49], hp[:, :, 49:98], OP.add)
            nv.copy_predicated(cell_t[:], obj_t[:], inner)
            nv.tensor_reduce(acc[:, c:c + 1], cell_t[:],
                             mybir.AxisListType.X, OP.add)

        nc.sync.dma_start(out_ap, acc[:])


_NC_CACHE = None


def build_nc():
    global _NC_CACHE
    if _NC_CACHE is not None:
        return _NC_CACHE
    nc = bacc.Bacc(
        "TRN2",
        target_bir_lowering=False,
        debug=False,
        enable_asserts=False,
        num_devices=NCORES,
    )
    pred = nc.dram_tensor("pred", [BLOC, PREDW], F32, kind="ExternalInput")
    labs = nc.dram_tensor("labs", [BLOC, LABW], F32, kind="ExternalInput")
    out = nc.dram_tensor("out", [P, NCHUNK], F32, kind="ExternalOutput")
    with tile.TileContext(nc) as tc:
        _body(tc, pred.ap(), labs.ap(), out.ap())
    nc.compile()
    _NC_CACHE = nc
    return nc


def make_in_maps(pred, labels):
    pred = np.asarray(pred, dtype=np.float32)
    labels = np.asarray(labels, dtype=np.float32)
    pred2 = np.ascontiguousarray(pred[:, PERM]).reshape(B, PREDW)
    # labels: [lx lx ly ly | lw lw lh lh | obj | cls] (gt dup'd per box)
    lab2 = np.ascontiguousarray(
        labels[:, [0, 0, 1, 1, 2, 2, 3, 3, 4] + list(range(10, 30))]
    ).reshape(B, LABW)
    return [
        {
            "pred": np.ascontiguousarray(pred2[i * BLOC:(i + 1) * BLOC]),
            "labs": np.ascontiguousarray(lab2[i * BLOC:(i + 1) * BLOC]),
        }
        for i in range(NCORES)
    ]


def run(pred, labels, trace=False, **kw):
    nc = build_nc()
    in_maps = make_in_maps(pred, labels)
    res = run_bass_kernel_spmd(
        nc, in_maps, core_ids=list(range(NCORES)), trace=trace, **kw)
    total = np.float64(0.0)
    for r in res.results:
        total += r["out"].astype(np.float64).sum()
    loss = np.float32(total / B)
    return loss, res


def kernel(pred, labels):
    loss, _ = run(pred, labels)
    return np.array(loss, dtype=np.float32)
